# revision 27
# baseline (speedup 1.0000x reference)
"""Trainium2 Bass kernel for the SE(3) deformation model (v3).

reference math (per point):
    w, v, pivot, t = split(network_output, 4)
    theta = |w| + eps ; wn = w/theta ; vn = v/theta
    R = I + sin(theta) K + (1-cos(theta)) K^2          (K = skew(wn))
    p = (theta I + (1-cos) K + (theta-sin) K^2) vn
    out = R (x + pivot) + p - pivot + t - x

Exact rewrite used here (K~ = skew(w) unnormalized, n2 = |w|^2):
    u  = x + pivot ; a = v + t
    k1 = sin(th)/th ; k2 = (1-cos th)/th^2 ; sg = (th-sin th)/th^3
    g  = k1 u + k2 v ; h = k2 u + sg v
    out = (w x g) + (w.h) w - n2 h + a
(K~^2 y = w (w.y) - n2 y; algebraically identical to the reference.)

v3 design notes (minimize TOTAL engine time; concurrent-engine SBUF
contention taxes every op, so total work matters more than balance):
  - sin and 1-cos are degree-9 polynomial custom DVE ops (range-reduced
    by ADD_RANGE_WRAP).  ACT then only needs Copy/Square/Sqrt, which
    share one activation table -> zero ACT table loads.
  - No PE path: the five-term sum is 3 DVE tensor ops via the paired
    [pos|m1] - [neg|m2] layout.
  - ACT: deinterleaves of w,v; w^2; sqrt; 1/th cast; extended-plane
    copies; final interleave.  GPSIMD: u, a, m2 (fused strided ops).
  - DVE: coefficient chain (fp16) + all wide fp16 muls at 2x mode.
  - Output stored as raw fp16 (HWDGE), upcast to f32 on the host.
"""

import math
import re

import numpy as np

import concourse.bacc as bacc
import concourse.mybir as mybir
import concourse.tile as tile
from concourse.alu_op_type import AluOpType
from concourse.bass_utils import run_bass_kernel_spmd

AFT = mybir.ActivationFunctionType
F32 = mybir.dt.float32
F16 = mybir.dt.float16

N_TOTAL = 4194304
NCORES = 8
NPC = N_TOTAL // NCORES  # 524288 points per core
P = 128
F_DEF = 1024
EPS = 1e-6
EPS2 = EPS * EPS

# ---------------------------------------------------------------- custom ops
from concourse import dve_ops as _dvo
from concourse.dve_spec import Spec, Src0, Src1, C0, C1, C2, sq as _sq


def _fit_odd_poly():
    # sin(x) ~ x*(a0 + a1 y + a2 y^2 + a3 y^3), y = x^2, |x| <= pi
    x = np.cos(np.linspace(0, np.pi, 4001)) * np.pi
    y = x * x
    A = np.stack([x, x * y, x * y * y, x * y * y * y], axis=1)
    a = np.linalg.lstsq(A, np.sin(x), rcond=None)[0]
    # 1-cos(x) ~ y*(b0 + b1 y + b2 y^2 + b3 y^3)
    B = np.stack([y, y * y, y * y * y, y * y * y * y], axis=1)
    b = np.linalg.lstsq(B, 1.0 - np.cos(x), rcond=None)[0]
    return [float(v) for v in a], [float(v) for v in b]


SIN_A, COS_B = _fit_odd_poly()


def _make_op(name, spec):
    op = _dvo.DveOp(name, spec, subdim=False, uops_sha={})
    if not any(o.name == op.name for o in _dvo.OPS):
        _dvo.OPS.append(op)
        _dvo.CUSTOM_DVE_SPECS[op.name] = op.spec
        _dvo._SUB_OPCODE_FOR_NAME[op.name] = (
            _dvo._CUSTOM_DVE_ROW_BASE + len(_dvo.OPS) - 1
        )
    for ver in ("v3", "v4"):
        try:
            op.compile(ver)
        except ValueError as e:
            m = re.search(r'uops_sha\["' + ver + r'"\]="([0-9a-f]+)"', str(e))
            if m:
                op.uops_sha[ver] = m.group(1)
            else:
                raise
    return op


_y = _sq(Src0)
SIN_POLY = _make_op(
    "SIN_POLY_SE3",
    Spec(
        body=Src0 * (C0 + _y * (C1 + _y * (C2 + _y * Src1))),
        reference=lambda in0, in1, s0, s1, imm2: (
            in0 * (s0 + in0 * in0 * (s1 + in0 * in0 * (imm2 + in0 * in0 * in1)))
        ).astype(np.float32),
    ),
)
ONE_MCOS_POLY = _make_op(
    "ONE_MCOS_SE3",
    Spec(
        body=_y * (C0 + _y * (C1 + _y * (C2 + _y * Src1))),
        reference=lambda in0, in1, s0, s1, imm2: (
            in0 * in0 * (s0 + in0 * in0 * (s1 + in0 * in0 * (imm2 + in0 * in0 * in1)))
        ).astype(np.float32),
    ),
)
# k2 = c1 * inv^2   (Src0=c1, Src1=inv)
MUL_SQ = _make_op(
    "MUL_SQ_SE3",
    Spec(
        body=Src0 * _sq(Src1),
        reference=lambda in0, in1, s0, s1, imm2: (in0 * in1 * in1).astype(np.float32),
    ),
)
# sg = thms * inv^3  (Src0=thms, Src1=inv)
MUL_CUBE = _make_op(
    "MUL_CUBE_SE3",
    Spec(
        body=Src0 * (Src1 * _sq(Src1)),
        reference=lambda in0, in1, s0, s1, imm2: (in0 * in1 * in1 * in1).astype(
            np.float32
        ),
    ),
)



def build_nc(npc: int = NPC, f: int = F_DEF):
    nchunks = npc // (P * f)
    assert nchunks * P * f == npc

    nc = bacc.Bacc("TRN2", target_bir_lowering=False, debug=False)

    # const AP for the Sqrt bias (non-Copy activation bias must be an AP)
    eps2_t = nc.alloc_sbuf_tensor("const-float32-eps2", [128, 1], F32)
    nc.gpsimd.memset(eps2_t.ap(), EPS2)
    nc.const_aps.aps[(F32, EPS2)] = eps2_t.ap()
    # full [P,f] tensors holding the poly cubic coefficients ([P,1]-bc Src1
    # crashes the DVE custom-op path; a full replicated tile works)
    sin_a3 = nc.alloc_sbuf_tensor("sin-a3", [128, f], F32)
    nc.gpsimd.memset(sin_a3.ap(), SIN_A[3])
    cos_b3 = nc.alloc_sbuf_tensor("cos-b3", [128, f], F32)
    nc.gpsimd.memset(cos_b3.ap(), COS_B[3])
    nc.all_engine_barrier()

    pos = nc.dram_tensor("pos", [npc, 3], F32, kind="ExternalInput")
    net = nc.dram_tensor("net", [npc, 12], F32, kind="ExternalInput")
    out = nc.dram_tensor("out", [npc, 3], F16, kind="ExternalOutput")

    pos_r = pos.ap().rearrange("(n p f) c -> n p (f c)", p=P, f=f)
    net_r = net.ap().rearrange("(n p f) c -> n p (f c)", p=P, f=f)
    out_r = out.ap().rearrange("(n p f) c -> n p (f c)", p=P, f=f)

    V = nc.vector
    G = nc.gpsimd
    S = nc.scalar
    mul, add, sub = AluOpType.mult, AluOpType.add, AluOpType.subtract

    with tile.TileContext(nc) as tc:
        with (
            tc.tile_pool(name="io", bufs=2) as io,
            tc.tile_pool(name="wk2", bufs=2) as wk2,
            tc.tile_pool(name="wk", bufs=1) as wk,
            tc.tile_pool(name="sc", bufs=1) as sc,
        ):
            def issue_loads(i):
                x16 = io.tile([P, 3 * f], F16, tag="x", name="x16")
                net16 = io.tile([P, 12 * f], F16, tag="net", name="net16")
                G.dma_start(out=x16[:], in_=pos_r[i])
                G.dma_start(out=net16[:, 0 : 6 * f], in_=net_r[i][:, 0 : 6 * f])
                G.dma_start(out=net16[:, 6 * f : 12 * f], in_=net_r[i][:, 6 * f : 12 * f])
                return x16, net16

            pending = issue_loads(0)
            for i in range(nchunks):
                x16, net16 = pending
                if i + 1 < nchunks:
                    pending = issue_loads(i + 1)

                netp = net16[:].rearrange("p (f c) -> p c f", c=12)
                xp = x16[:].rearrange("p (f c) -> p c f", c=3)

                # ---- tiles ----
                uv = wk2.tile([P, 6 * f], F16, tag="uv", name="uv")  # [u|v]
                whw = wk2.tile([P, 8 * f], F16, tag="whw", name="whw")  # [h|w0w1w2w0w1]
                gext = wk.tile([P, 5 * f], F16, tag="gext", name="gext")
                m12 = wk.tile([P, 6 * f], F16, tag="m12", name="m12")  # [k1u|k2v] -> [pos|m1]
                m34 = wk.tile([P, 6 * f], F16, tag="m34", name="m34")  # [k2u|sgv] -> [neg|m2]
                m3o = wk.tile([P, 3 * f], F16, tag="m3o", name="m3o")  # sq, pr, out
                av = wk.tile([P, 3 * f], F16, tag="av", name="av")  # a = v+t, then out
                oil = m3o
                coefs = wk.tile([P, 3 * f], F16, tag="coefs", name="coefs")  # [k1|k2|sg]
                whn2 = wk.tile([P, 2 * f], F16, tag="whn2", name="whn2")  # [wh|n2]

                def stile(tag, dt=F32):
                    return sc.tile([P, f], dt, tag=tag, name=tag + "_t")

                c1 = stile("c1", F16)
                s16 = stile("s16", F16)
                inv = stile("inv", F16)
                inv2 = stile("inv2", F16)
                thms = stile("thms", F16)
                th = stile("th")
                thw = stile("thw")
                inv32 = stile("inv32")

                u_pl = uv[:, 0 : 3 * f]
                v_pl = uv[:, 3 * f : 6 * f]
                h_pl = whw[:, 0 : 3 * f]
                w_pl = whw[:, 3 * f : 6 * f]
                g_pl = gext[:, 0 : 3 * f]

                def pl(t, c):
                    return t[:, c * f : (c + 1) * f]

                def v3(t):
                    return t.rearrange("p (c f) -> p c f", c=3)

                def bc3(s_ap):
                    return s_ap.unsqueeze(1).to_broadcast((P, 3, f))

                def bc23(s_ap):
                    return (
                        s_ap.rearrange("p (a g) -> p a g", a=2)
                        .unsqueeze(2)
                        .to_broadcast((P, 2, 3, f))
                    )

                def v23(t):
                    return t.rearrange("p (a c f) -> p a c f", a=2, c=3)

                # ---- deinterleave (ACT) / u,a (GPSIMD) ----
                S.activation(v3(w_pl), netp[:, 0:3, :], AFT.Copy)
                S.activation(v3(v_pl), netp[:, 3:6, :], AFT.Copy)
                G.tensor_tensor(v3(u_pl), xp, netp[:, 6:9, :], add)
                G.tensor_tensor(v3(av[:]), v3(v_pl), netp[:, 9:12, :], add)

                # ---- sq (ACT) + n2 tree (m3o holds sq now, pr later) ----
                sqv = m3o[:, 0 : 3 * f]
                S.activation(v3(sqv), v3(w_pl), AFT.Square)
                V.tensor_tensor(whn2[:, f : 2 * f], pl(sqv, 0), pl(sqv, 1), add)
                V.tensor_tensor(whn2[:, f : 2 * f], whn2[:, f : 2 * f], pl(sqv, 2), add)

                # ---- scalar chain ----
                S.activation(th[:], whn2[:, f : 2 * f], AFT.Sqrt, bias=EPS2)
                V.reciprocal_approx_fast(out=inv32[:], in_=th[:])
                V.add_range_wrap(thw[:], th[:], 0.0, math.pi, 2 * math.pi)
                V._custom_dve(
                    SIN_POLY, out=s16[:], in0=thw[:], in1=sin_a3.ap(),
                    s0=SIN_A[0], s1=SIN_A[1], imm2=SIN_A[2],
                )
                V._custom_dve(
                    ONE_MCOS_POLY, out=c1[:], in0=thw[:], in1=cos_b3.ap(),
                    s0=COS_B[0], s1=COS_B[1], imm2=COS_B[2],
                )
                S.activation(inv[:], inv32[:], AFT.Copy)
                V.tensor_tensor(thms[:], th[:], s16[:], sub)
                V.tensor_tensor(inv2[:], inv[:], inv[:], mul)
                V.tensor_tensor(coefs[:, 0:f], s16[:], inv[:], mul)  # k1
                V.tensor_tensor(coefs[:, f : 2 * f], c1[:], inv2[:], mul)  # k2
                V.tensor_tensor(thms[:], thms[:], inv[:], mul)
                V.tensor_tensor(thms[:], thms[:], inv[:], mul)
                V.tensor_tensor(coefs[:, 2 * f : 3 * f], thms[:], inv[:], mul)  # sg

                # ---- g = k1 u + k2 v ; h = k2 u + sg v ----
                V.tensor_tensor(v23(m12[:]), v23(uv[:]), bc23(coefs[:, 0 : 2 * f]), mul)
                V.tensor_tensor(g_pl, m12[:, 0 : 3 * f], m12[:, 3 * f : 6 * f], add)
                V.tensor_tensor(v23(m34[:]), v23(uv[:]), bc23(coefs[:, f : 3 * f]), mul)
                V.tensor_tensor(h_pl, m34[:, 0 : 3 * f], m34[:, 3 * f : 6 * f], add)

                # ---- pr = w.h planes ; wh tree ----
                V.tensor_tensor(v3(m3o[:, 0 : 3 * f]), v3(h_pl), v3(w_pl), mul)
                V.tensor_tensor(whn2[:, 0:f], pl(m3o, 0), pl(m3o, 1), add)
                V.tensor_tensor(whn2[:, 0:f], whn2[:, 0:f], pl(m3o, 2), add)

                # ---- extended planes (ACT) + cross products ----
                S.activation(whw[:, 6 * f : 8 * f], whw[:, 3 * f : 5 * f], AFT.Copy)
                S.activation(gext[:, 3 * f : 5 * f], gext[:, 0 : 2 * f], AFT.Copy)
                V.tensor_tensor(
                    m12[:, 0 : 3 * f], whw[:, 4 * f : 7 * f], gext[:, 2 * f : 5 * f], mul
                )  # pos
                V.tensor_tensor(
                    m34[:, 0 : 3 * f], whw[:, 5 * f : 8 * f], gext[:, f : 4 * f], mul
                )  # neg

                # ---- m1 = wh*w (DVE) ; m2 = n2*h (GPSIMD) ----
                V.tensor_tensor(
                    v3(m12[:, 3 * f : 6 * f]), v3(w_pl), bc3(whn2[:, 0:f]), mul
                )
                G.tensor_tensor(
                    v3(m34[:, 3 * f : 6 * f]), v3(h_pl), bc3(whn2[:, f : 2 * f]), mul
                )

                # ---- sum: out = (pos-neg) + (m1-m2) + a ----
                V.tensor_tensor(m12[:], m12[:], m34[:], sub)  # [pos-neg | m1-m2]
                V.tensor_tensor(
                    m34[:, 0 : 3 * f], m12[:, 0 : 3 * f], m12[:, 3 * f : 6 * f], add
                )
                V.tensor_tensor(av[:], m34[:, 0 : 3 * f], av[:], add)

                # ---- interleave (ACT) + store raw fp16 ----
                o_il = oil[:].rearrange("p (f c) -> p c f", c=3)
                S.activation(o_il, v3(av[:]), AFT.Copy)
                nc.sync.dma_start(out=out_r[i], in_=oil[:])

    nc.compile()
    return nc


_NC_CACHE: dict = {}


def _get_nc():
    if "nc" not in _NC_CACHE:
        _NC_CACHE["nc"] = build_nc()
    return _NC_CACHE["nc"]


def make_in_maps(pos: np.ndarray, net: np.ndarray):
    return [
        {
            "pos": pos[i * NPC : (i + 1) * NPC],
            "net": net[i * NPC : (i + 1) * NPC],
        }
        for i in range(NCORES)
    ]


def kernel(undeformed_positions: np.ndarray, network_output: np.ndarray) -> np.ndarray:
    pos = np.ascontiguousarray(np.asarray(undeformed_positions, dtype=np.float32))
    net = np.ascontiguousarray(np.asarray(network_output, dtype=np.float32))
    assert pos.shape == (N_TOTAL, 3) and net.shape == (N_TOTAL, 12)

    nc = _get_nc()
    res = run_bass_kernel_spmd(nc, make_in_maps(pos, net), list(range(NCORES)))
    return np.concatenate(
        [res.results[i]["out"].astype(np.float32) for i in range(NCORES)], axis=0
    )


# revision 28
# speedup vs baseline: 1.1169x; 1.1169x over previous
"""Trainium2 Bass kernel for the SE(3) deformation model (v3).

reference math (per point):
    w, v, pivot, t = split(network_output, 4)
    theta = |w| + eps ; wn = w/theta ; vn = v/theta
    R = I + sin(theta) K + (1-cos(theta)) K^2          (K = skew(wn))
    p = (theta I + (1-cos) K + (theta-sin) K^2) vn
    out = R (x + pivot) + p - pivot + t - x

Exact rewrite used here (K~ = skew(w) unnormalized, n2 = |w|^2):
    u  = x + pivot ; a = v + t
    k1 = sin(th)/th ; k2 = (1-cos th)/th^2 ; sg = (th-sin th)/th^3
    g  = k1 u + k2 v ; h = k2 u + sg v
    out = (w x g) + (w.h) w - n2 h + a
(K~^2 y = w (w.y) - n2 y; algebraically identical to the reference.)

v3 design notes (minimize TOTAL engine time; concurrent-engine SBUF
contention taxes every op, so total work matters more than balance):
  - sin and 1-cos are degree-9 polynomial custom DVE ops (range-reduced
    by ADD_RANGE_WRAP).  ACT then only needs Copy/Square/Sqrt, which
    share one activation table -> zero ACT table loads.
  - No PE path: the five-term sum is 3 DVE tensor ops via the paired
    [pos|m1] - [neg|m2] layout.
  - ACT: deinterleaves of w,v; w^2; sqrt; 1/th cast; extended-plane
    copies; final interleave.  GPSIMD: u, a, m2 (fused strided ops).
  - DVE: coefficient chain (fp16) + all wide fp16 muls at 2x mode.
  - Output stored as raw fp16 (HWDGE), upcast to f32 on the host.
"""

import math
import re

import numpy as np

import concourse.bacc as bacc
import concourse.mybir as mybir
import concourse.tile as tile
from concourse.alu_op_type import AluOpType
from concourse.bass_utils import run_bass_kernel_spmd

AFT = mybir.ActivationFunctionType
F32 = mybir.dt.float32
F16 = mybir.dt.float16

N_TOTAL = 4194304
NCORES = 8
NPC = N_TOTAL // NCORES  # 524288 points per core
P = 128
F_DEF = 512
EPS = 1e-6
EPS2 = EPS * EPS

# ---------------------------------------------------------------- custom ops
from concourse import dve_ops as _dvo
from concourse.dve_spec import Spec, Src0, Src1, C0, C1, C2, sq as _sq


def _fit_odd_poly():
    # sin(x) ~ x*(a0 + a1 y + a2 y^2 + a3 y^3), y = x^2, |x| <= pi
    x = np.cos(np.linspace(0, np.pi, 4001)) * np.pi
    y = x * x
    A = np.stack([x, x * y, x * y * y, x * y * y * y], axis=1)
    a = np.linalg.lstsq(A, np.sin(x), rcond=None)[0]
    # 1-cos(x) ~ y*(b0 + b1 y + b2 y^2 + b3 y^3)
    B = np.stack([y, y * y, y * y * y, y * y * y * y], axis=1)
    b = np.linalg.lstsq(B, 1.0 - np.cos(x), rcond=None)[0]
    return [float(v) for v in a], [float(v) for v in b]


SIN_A, COS_B = _fit_odd_poly()


def _make_op(name, spec):
    op = _dvo.DveOp(name, spec, subdim=False, uops_sha={})
    if not any(o.name == op.name for o in _dvo.OPS):
        _dvo.OPS.append(op)
        _dvo.CUSTOM_DVE_SPECS[op.name] = op.spec
        _dvo._SUB_OPCODE_FOR_NAME[op.name] = (
            _dvo._CUSTOM_DVE_ROW_BASE + len(_dvo.OPS) - 1
        )
    for ver in ("v3", "v4"):
        try:
            op.compile(ver)
        except ValueError as e:
            m = re.search(r'uops_sha\["' + ver + r'"\]="([0-9a-f]+)"', str(e))
            if m:
                op.uops_sha[ver] = m.group(1)
            else:
                raise
    return op


_y = _sq(Src0)
SIN_POLY = _make_op(
    "SIN_POLY_SE3",
    Spec(
        body=Src0 * (C0 + _y * (C1 + _y * (C2 + _y * Src1))),
        reference=lambda in0, in1, s0, s1, imm2: (
            in0 * (s0 + in0 * in0 * (s1 + in0 * in0 * (imm2 + in0 * in0 * in1)))
        ).astype(np.float32),
    ),
)
ONE_MCOS_POLY = _make_op(
    "ONE_MCOS_SE3",
    Spec(
        body=_y * (C0 + _y * (C1 + _y * (C2 + _y * Src1))),
        reference=lambda in0, in1, s0, s1, imm2: (
            in0 * in0 * (s0 + in0 * in0 * (s1 + in0 * in0 * (imm2 + in0 * in0 * in1)))
        ).astype(np.float32),
    ),
)
# k2 = c1 * inv^2   (Src0=c1, Src1=inv)
MUL_SQ = _make_op(
    "MUL_SQ_SE3",
    Spec(
        body=Src0 * _sq(Src1),
        reference=lambda in0, in1, s0, s1, imm2: (in0 * in1 * in1).astype(np.float32),
    ),
)
# sg = thms * inv^3  (Src0=thms, Src1=inv)
MUL_CUBE = _make_op(
    "MUL_CUBE_SE3",
    Spec(
        body=Src0 * (Src1 * _sq(Src1)),
        reference=lambda in0, in1, s0, s1, imm2: (in0 * in1 * in1 * in1).astype(
            np.float32
        ),
    ),
)



def build_nc(npc: int = NPC, f: int = F_DEF):
    nchunks = npc // (P * f)
    assert nchunks * P * f == npc

    nc = bacc.Bacc("TRN2", target_bir_lowering=False, debug=False)

    # const AP for the Sqrt bias (non-Copy activation bias must be an AP)
    eps2_t = nc.alloc_sbuf_tensor("const-float32-eps2", [128, 1], F32)
    nc.gpsimd.memset(eps2_t.ap(), EPS2)
    nc.const_aps.aps[(F32, EPS2)] = eps2_t.ap()
    # full [P,f] tensors holding the poly cubic coefficients ([P,1]-bc Src1
    # crashes the DVE custom-op path; a full replicated tile works)
    sin_a3 = nc.alloc_sbuf_tensor("sin-a3", [128, f], F32)
    nc.gpsimd.memset(sin_a3.ap(), SIN_A[3])
    cos_b3 = nc.alloc_sbuf_tensor("cos-b3", [128, f], F32)
    nc.gpsimd.memset(cos_b3.ap(), COS_B[3])
    nc.all_engine_barrier()

    pos = nc.dram_tensor("pos", [npc, 3], F32, kind="ExternalInput")
    net = nc.dram_tensor("net", [npc, 12], F32, kind="ExternalInput")
    out = nc.dram_tensor("out", [npc, 3], F16, kind="ExternalOutput")

    pos_r = pos.ap().rearrange("(n p f) c -> n p (f c)", p=P, f=f)
    net_r = net.ap().rearrange("(n p f) c -> n p (f c)", p=P, f=f)
    out_r = out.ap().rearrange("(n p f) c -> n p (f c)", p=P, f=f)

    V = nc.vector
    G = nc.gpsimd
    S = nc.scalar
    mul, add, sub = AluOpType.mult, AluOpType.add, AluOpType.subtract

    with tile.TileContext(nc) as tc:
        with (
            tc.tile_pool(name="io", bufs=3) as io,
            tc.tile_pool(name="wk", bufs=2) as wk,
            tc.tile_pool(name="sc", bufs=2) as sc,
        ):
            def issue_loads(i):
                x16 = io.tile([P, 3 * f], F16, tag="x", name="x16")
                net16 = io.tile([P, 12 * f], F16, tag="net", name="net16")
                G.dma_start(out=x16[:], in_=pos_r[i])
                G.dma_start(out=net16[:, 0 : 6 * f], in_=net_r[i][:, 0 : 6 * f])
                G.dma_start(out=net16[:, 6 * f : 12 * f], in_=net_r[i][:, 6 * f : 12 * f])
                return x16, net16

            pending = issue_loads(0)
            for i in range(nchunks):
                x16, net16 = pending
                if i + 1 < nchunks:
                    pending = issue_loads(i + 1)

                netp = net16[:].rearrange("p (f c) -> p c f", c=12)
                xp = x16[:].rearrange("p (f c) -> p c f", c=3)

                # ---- tiles ----
                uv = wk.tile([P, 6 * f], F16, tag="uv", name="uv")  # [u|v]
                whw = wk.tile([P, 8 * f], F16, tag="whw", name="whw")  # [h|w0w1w2w0w1]
                gext = wk.tile([P, 5 * f], F16, tag="gext", name="gext")
                m12 = wk.tile([P, 6 * f], F16, tag="m12", name="m12")  # [k1u|k2v] -> [pos|m1]
                m34 = wk.tile([P, 6 * f], F16, tag="m34", name="m34")  # [k2u|sgv] -> [neg|m2]
                m3o = wk.tile([P, 6 * f], F16, tag="m3o", name="m3o")  # [pr|sq]
                av = wk.tile([P, 3 * f], F16, tag="av", name="av")  # a = v+t
                o16 = wk.tile([P, 3 * f], F16, tag="o", name="o16")
                oil = wk.tile([P, 3 * f], F16, tag="oil", name="oil")
                coefs = wk.tile([P, 3 * f], F16, tag="coefs", name="coefs")  # [k1|k2|sg]
                whn2 = wk.tile([P, 2 * f], F16, tag="whn2", name="whn2")  # [wh|n2]

                def stile(tag, dt=F32):
                    return sc.tile([P, f], dt, tag=tag, name=tag + "_t")

                c1 = stile("c1", F16)
                s16 = stile("s16", F16)
                inv = stile("inv", F16)
                inv2 = stile("inv2", F16)
                thms = stile("thms", F16)
                tsg = stile("tsg", F16)
                th = stile("th")
                thw = stile("thw")
                inv32 = stile("inv32")

                u_pl = uv[:, 0 : 3 * f]
                v_pl = uv[:, 3 * f : 6 * f]
                h_pl = whw[:, 0 : 3 * f]
                w_pl = whw[:, 3 * f : 6 * f]
                g_pl = gext[:, 0 : 3 * f]

                def pl(t, c):
                    return t[:, c * f : (c + 1) * f]

                def v3(t):
                    return t.rearrange("p (c f) -> p c f", c=3)

                def bc3(s_ap):
                    return s_ap.unsqueeze(1).to_broadcast((P, 3, f))

                def bc23(s_ap):
                    return (
                        s_ap.rearrange("p (a g) -> p a g", a=2)
                        .unsqueeze(2)
                        .to_broadcast((P, 2, 3, f))
                    )

                def v23(t):
                    return t.rearrange("p (a c f) -> p a c f", a=2, c=3)

                # ---- deinterleave (ACT) / u,a (GPSIMD) ----
                S.activation(v3(w_pl), netp[:, 0:3, :], AFT.Copy)
                S.activation(v3(v_pl), netp[:, 3:6, :], AFT.Copy)
                G.tensor_tensor(v3(u_pl), xp, netp[:, 6:9, :], add)
                G.tensor_tensor(v3(av[:]), v3(v_pl), netp[:, 9:12, :], add)

                # ---- sq (ACT) + n2 tree ----
                sqv = m3o[:, 3 * f : 6 * f]
                S.activation(v3(sqv), v3(w_pl), AFT.Square)
                V.tensor_tensor(whn2[:, f : 2 * f], pl(sqv, 0), pl(sqv, 1), add)
                V.tensor_tensor(whn2[:, f : 2 * f], whn2[:, f : 2 * f], pl(sqv, 2), add)

                # ---- scalar chain ----
                S.activation(th[:], whn2[:, f : 2 * f], AFT.Sqrt, bias=EPS2)
                V.reciprocal_approx_fast(out=inv32[:], in_=th[:])
                V.add_range_wrap(thw[:], th[:], 0.0, math.pi, 2 * math.pi)
                V._custom_dve(
                    SIN_POLY, out=s16[:], in0=thw[:], in1=sin_a3.ap(),
                    s0=SIN_A[0], s1=SIN_A[1], imm2=SIN_A[2],
                )
                V._custom_dve(
                    ONE_MCOS_POLY, out=c1[:], in0=thw[:], in1=cos_b3.ap(),
                    s0=COS_B[0], s1=COS_B[1], imm2=COS_B[2],
                )
                S.activation(inv[:], inv32[:], AFT.Copy)
                V.tensor_tensor(thms[:], th[:], s16[:], sub)
                V.tensor_tensor(inv2[:], inv[:], inv[:], mul)
                V.tensor_tensor(coefs[:, 0:f], s16[:], inv[:], mul)  # k1
                V.tensor_tensor(coefs[:, f : 2 * f], c1[:], inv2[:], mul)  # k2
                V.tensor_tensor(tsg[:], thms[:], inv[:], mul)
                V.tensor_tensor(tsg[:], tsg[:], inv[:], mul)
                V.tensor_tensor(coefs[:, 2 * f : 3 * f], tsg[:], inv[:], mul)  # sg

                # ---- g = k1 u + k2 v ; h = k2 u + sg v ----
                V.tensor_tensor(v23(m12[:]), v23(uv[:]), bc23(coefs[:, 0 : 2 * f]), mul)
                V.tensor_tensor(g_pl, m12[:, 0 : 3 * f], m12[:, 3 * f : 6 * f], add)
                V.tensor_tensor(v23(m34[:]), v23(uv[:]), bc23(coefs[:, f : 3 * f]), mul)
                V.tensor_tensor(h_pl, m34[:, 0 : 3 * f], m34[:, 3 * f : 6 * f], add)

                # ---- pr = w.h planes ; wh tree ----
                V.tensor_tensor(v3(m3o[:, 0 : 3 * f]), v3(h_pl), v3(w_pl), mul)
                V.tensor_tensor(whn2[:, 0:f], pl(m3o, 0), pl(m3o, 1), add)
                V.tensor_tensor(whn2[:, 0:f], whn2[:, 0:f], pl(m3o, 2), add)

                # ---- extended planes (ACT) + cross products ----
                S.activation(whw[:, 6 * f : 8 * f], whw[:, 3 * f : 5 * f], AFT.Copy)
                S.activation(gext[:, 3 * f : 5 * f], gext[:, 0 : 2 * f], AFT.Copy)
                V.tensor_tensor(
                    m12[:, 0 : 3 * f], whw[:, 4 * f : 7 * f], gext[:, 2 * f : 5 * f], mul
                )  # pos
                V.tensor_tensor(
                    m34[:, 0 : 3 * f], whw[:, 5 * f : 8 * f], gext[:, f : 4 * f], mul
                )  # neg

                # ---- m1 = wh*w (DVE) ; m2 = n2*h (GPSIMD) ----
                V.tensor_tensor(
                    v3(m12[:, 3 * f : 6 * f]), v3(w_pl), bc3(whn2[:, 0:f]), mul
                )
                G.tensor_tensor(
                    v3(m34[:, 3 * f : 6 * f]), v3(h_pl), bc3(whn2[:, f : 2 * f]), mul
                )

                # ---- sum: out = (pos-neg) + (m1-m2) + a ----
                V.tensor_tensor(m12[:], m12[:], m34[:], sub)  # [pos-neg | m1-m2]
                V.tensor_tensor(o16[:], m12[:, 0 : 3 * f], m12[:, 3 * f : 6 * f], add)
                V.tensor_tensor(o16[:], o16[:], av[:], add)

                # ---- interleave (ACT) + store raw fp16 ----
                o_il = oil[:].rearrange("p (f c) -> p c f", c=3)
                S.activation(o_il, v3(o16[:]), AFT.Copy)
                nc.sync.dma_start(out=out_r[i], in_=oil[:])

    nc.compile()
    return nc


_NC_CACHE: dict = {}


def _get_nc():
    if "nc" not in _NC_CACHE:
        _NC_CACHE["nc"] = build_nc()
    return _NC_CACHE["nc"]


def make_in_maps(pos: np.ndarray, net: np.ndarray):
    return [
        {
            "pos": pos[i * NPC : (i + 1) * NPC],
            "net": net[i * NPC : (i + 1) * NPC],
        }
        for i in range(NCORES)
    ]


def kernel(undeformed_positions: np.ndarray, network_output: np.ndarray) -> np.ndarray:
    pos = np.ascontiguousarray(np.asarray(undeformed_positions, dtype=np.float32))
    net = np.ascontiguousarray(np.asarray(network_output, dtype=np.float32))
    assert pos.shape == (N_TOTAL, 3) and net.shape == (N_TOTAL, 12)

    nc = _get_nc()
    res = run_bass_kernel_spmd(nc, make_in_maps(pos, net), list(range(NCORES)))
    return np.concatenate(
        [res.results[i]["out"].astype(np.float32) for i in range(NCORES)], axis=0
    )
